# revision 7
# baseline (speedup 1.0000x reference)
# Gemma2 sliding-window attention on 8 Trainium2 NeuronCores (Bass/Tile).
#
# Sharding: core c -> (batch = c//2, head-group = c%2).  Each core computes
# 4 q-heads / 2 kv-heads of one batch: QKV projections, RoPE, windowed
# causal attention, and a partial output projection y_part = attn @ Wo[rows
# of its head-group].  The host sums the two partials per batch (Wo is
# split on its input dim) — that is the only cross-core reduction.
#
# On-device layout strategy (all matmuls bf16, accumulation fp32 in PSUM):
#   hsT  [H, S]    host-pre-transposed hidden states (bf16)
#   qT   [dq, S]   = Wq_chunk.T @ hsT   (RoPE fused into PSUM evacuation)
#   kT   [dk, S]   likewise
#   v    [S, dk]   = hsT_chunk.T @ Wv
#   scoresT [kj, qi] = kT_chunk.T @ qT  (transposed scores: feeds PV directly)
#   expT = exp(scoresT * SCALE)         (no running max: |scores*SCALE| < ~7)
#   attnT [d, qi] = v_chunk.T @ expT ; sums[1, qi] = ones.T @ expT
#   attnT normalized by broadcast(1/sums) at PSUM evacuation
#   y [S, H] = attnT_chunk.T @ Wo_chunk
import os
import sys

import numpy as np
import ml_dtypes

bf16 = ml_dtypes.bfloat16

B, S, H = 4, 2048, 2048
NH, NKV, HD = 8, 4, 256
WINDOW = 1024
ROPE_BASE = 10000.0
SCALE = 256.0 ** -0.5
NCORES = 8

NHC, NKVC = NH // 2, NKV // 2      # per-core q heads / kv heads
DQ, DK = NHC * HD, NKVC * HD       # 1024, 512
HALF = HD // 2                     # 128 (rope half-dim == d-chunk)
NQD, NKD = DQ // 128, DK // 128    # q / k d-blocks (8 / 4)
NHCH = H // 128                    # 16 hidden chunks
TB = 512                           # token block for projections
NTB = S // TB                      # 4
QW = 256                           # attention query-block width
NQB = S // QW                      # 8
NKB = S // 128                     # 16 key blocks
# boundary-mask IDs by D = qi0 - kj0
MASK_D = {0: 0, -128: 1, 896: 2, 1024: 3}

_PROGRAM_CACHE = {}


def _ensure_paths():
    try:
        import concourse.bass  # noqa: F401
    except ImportError:
        for p in ("/root/.axon_site/_ro/trn_rl_repo", "/opt/trn_rl_repo"):
            if os.path.isdir(p) and p not in sys.path:
                sys.path.insert(0, p)


def _build_program():
    _ensure_paths()
    import concourse.bacc as bacc
    import concourse.bass as bass
    import concourse.mybir as mybir
    import concourse.tile as tile
    from contextlib import ExitStack

    f32 = mybir.dt.float32
    b16 = mybir.dt.bfloat16
    Alu = mybir.AluOpType
    Act = mybir.ActivationFunctionType
    ts = bass.ts

    nc = bacc.Bacc("TRN2", target_bir_lowering=False, debug=False,
                   num_devices=NCORES)

    hsT_d = nc.dram_tensor("hsT", [H, S], b16, kind="ExternalInput").ap()
    wq_d = nc.dram_tensor("wq", [H, DQ], b16, kind="ExternalInput").ap()
    wk_d = nc.dram_tensor("wk", [H, DK], b16, kind="ExternalInput").ap()
    wv_d = nc.dram_tensor("wv", [H, DK], b16, kind="ExternalInput").ap()
    wo_d = nc.dram_tensor("wo", [DQ, H], b16, kind="ExternalInput").ap()
    cos_d = nc.dram_tensor("cosT", [HALF, S], f32, kind="ExternalInput").ap()
    sin_d = nc.dram_tensor("sinT", [HALF, S], f32, kind="ExternalInput").ap()
    masks_d = nc.dram_tensor("masks", [128, 4 * QW], b16, kind="ExternalInput").ap()
    bq_d = nc.dram_tensor("bq", [128, NQD], f32, kind="ExternalInput").ap()
    bk_d = nc.dram_tensor("bk", [128, NKD], f32, kind="ExternalInput").ap()
    bv_d = nc.dram_tensor("bv", [1, DK], b16, kind="ExternalInput").ap()
    y_d = nc.dram_tensor("y", [S, H], b16, kind="ExternalOutput").ap()

    with tile.TileContext(nc) as tc, ExitStack() as ctx:
        persist = ctx.enter_context(tc.tile_pool(name="persist", bufs=1))

        # ---- persistent small constants -------------------------------
        masks_sb = persist.tile([128, 4 * QW], b16, tag="masks")
        nc.sync.dma_start(masks_sb[:], masks_d[:])
        bq_sb = persist.tile([128, NQD], f32, tag="bq")
        nc.sync.dma_start(bq_sb[:], bq_d[:])
        bk_sb = persist.tile([128, NKD], f32, tag="bk")
        nc.sync.dma_start(bk_sb[:], bk_d[:])
        bv_sb = persist.tile([1, DK], b16, tag="bv")
        nc.sync.dma_start(bv_sb[:], bv_d[:])
        ones_col = persist.tile([128, 1], b16, tag="ones_col")
        nc.gpsimd.memset(ones_col[:], 1.0)
        ones_rowb = persist.tile([1, 128], b16, tag="ones_rowb")
        nc.gpsimd.memset(ones_rowb[:], 1.0)
        ones_rowf = persist.tile([1, 128], f32, tag="ones_rowf")
        nc.gpsimd.memset(ones_rowf[:], 1.0)

        # ---- persistent activation arrays -----------------------------
        qT = [persist.tile([128, S], b16, tag=f"qT{i}", name=f"qT{i}")
              for i in range(NQD)]
        kT = [persist.tile([128, S], b16, tag=f"kT{i}", name=f"kT{i}")
              for i in range(NKD)]
        v_sb = [persist.tile([128, DK], b16, tag=f"v{i}", name=f"v{i}")
                for i in range(NKB)]

        # ================= phase 1: QKV projections ====================
        with tc.tile_pool(name="qkvw", bufs=1) as wpool, \
             tc.tile_pool(name="hst", bufs=24) as hpool, \
             tc.tile_pool(name="ropetmp", bufs=6) as tpool, \
             tc.tile_pool(name="ps_qkv", bufs=4, space="PSUM") as ps_qkv:

            cos_sb = wpool.tile([HALF, S], f32, tag="cos")
            nc.sync.dma_start(cos_sb[:], cos_d[:])
            sin_sb = wpool.tile([HALF, S], f32, tag="sin")
            nc.sync.dma_start(sin_sb[:], sin_d[:])

            wq_sb, wk_sb, wv_sb = [], [], []
            for hc in range(NHCH):
                t = wpool.tile([128, DQ], b16, tag=f"wq{hc}")
                nc.sync.dma_start(t[:], wq_d[ts(hc, 128), :])
                wq_sb.append(t)
                t = wpool.tile([128, DK], b16, tag=f"wk{hc}")
                nc.sync.dma_start(t[:], wk_d[ts(hc, 128), :])
                wk_sb.append(t)
                t = wpool.tile([128, DK], b16, tag=f"wv{hc}")
                nc.sync.dma_start(t[:], wv_d[ts(hc, 128), :])
                wv_sb.append(t)

            for tb in range(NTB):
                hsT_sb = []
                for hc in range(NHCH):
                    t = hpool.tile([128, TB], b16, tag="hst")
                    nc.sync.dma_start(t[:], hsT_d[ts(hc, 128), ts(tb, TB)])
                    hsT_sb.append(t)

                # q / k projections, d-block pairs, rope on evacuation
                for kind, nd, w_sb, out_sb, b_sb in (
                        ("q", NQD, wq_sb, qT, bq_sb),
                        ("k", NKD, wk_sb, kT, bk_sb)):
                    for dp in range(nd // 2):
                        ps = []
                        for c in range(2):
                            db = 2 * dp + c
                            p = ps_qkv.tile([128, TB], f32, tag="psqkv")
                            for hc in range(NHCH):
                                nc.tensor.matmul(
                                    p[:], w_sb[hc][:, ts(db, 128)],
                                    hsT_sb[hc][:],
                                    start=(hc == 0), stop=(hc == NHCH - 1))
                            ps.append(p)
                        # rope: out1 = (p0+b0)c - (p1+b1)s ; out2 = (p1+b1)c + (p0+b0)s
                        b0 = b_sb[:, 2 * dp:2 * dp + 1]
                        b1 = b_sb[:, 2 * dp + 1:2 * dp + 2]
                        cs = cos_sb[:, ts(tb, TB)]
                        sn = sin_sb[:, ts(tb, TB)]
                        t1 = tpool.tile([128, TB], f32, tag="rt")
                        nc.vector.scalar_tensor_tensor(
                            t1[:], ps[0][:], b0, cs, Alu.add, Alu.mult)
                        t2 = tpool.tile([128, TB], f32, tag="rt")
                        nc.vector.scalar_tensor_tensor(
                            t2[:], ps[1][:], b1, sn, Alu.add, Alu.mult)
                        nc.vector.tensor_sub(
                            out_sb[2 * dp][:, ts(tb, TB)], t1[:], t2[:])
                        t3 = tpool.tile([128, TB], f32, tag="rt")
                        nc.vector.scalar_tensor_tensor(
                            t3[:], ps[1][:], b1, cs, Alu.add, Alu.mult)
                        t4 = tpool.tile([128, TB], f32, tag="rt")
                        nc.vector.scalar_tensor_tensor(
                            t4[:], ps[0][:], b0, sn, Alu.add, Alu.mult)
                        nc.vector.tensor_add(
                            out_sb[2 * dp + 1][:, ts(tb, TB)], t3[:], t4[:])

                # v projection (token-major) + bias row
                for st in range(TB // 128):
                    kb = tb * (TB // 128) + st
                    p = ps_qkv.tile([128, DK], f32, tag="psqkv")
                    for hc in range(NHCH):
                        nc.tensor.matmul(
                            p[:], hsT_sb[hc][:, ts(st, 128)], wv_sb[hc][:],
                            start=(hc == 0), stop=False)
                    nc.tensor.matmul(p[:], ones_rowb[:], bv_sb[:],
                                     start=False, stop=True)
                    nc.scalar.activation(v_sb[kb][:], p[:], Act.Copy)

        # ============ phase 2: windowed attention per head =============
        attnT = [[[None, None] for _ in range(NQB)] for _ in range(NHC)]
        with tc.tile_pool(name="attn", bufs=1) as apool:
            wo_sb = []
            for dc in range(NQD):
                t = apool.tile([128, H], b16, tag=f"wo{dc}")
                nc.sync.dma_start(t[:], wo_d[ts(dc, 128), :])
                wo_sb.append(t)

            with tc.tile_pool(name="expp", bufs=12) as epool, \
                 tc.tile_pool(name="ps_sc", bufs=3, space="PSUM") as ps_sc, \
                 tc.tile_pool(name="ps_at", bufs=3, space="PSUM") as ps_at, \
                 tc.tile_pool(name="ps_sum", bufs=2, space="PSUM") as ps_sum:
                for h in range(NHC):
                    kv = h // 2
                    for qb in range(NQB):
                        qi0 = qb * QW
                        t_lo = max(0, 2 * qb - 8)
                        t_hi = 2 * qb + 1
                        sums = ps_sum.tile([1, QW], f32, tag="sums")
                        at = [ps_at.tile([128, QW], f32, tag="at", name=f"at{h}_{qb}_{c}")
                              for c in range(2)]
                        for t in range(t_lo, t_hi + 1):
                            sc = ps_sc.tile([128, QW], f32, tag="sc")
                            for c in range(2):
                                nc.tensor.matmul(
                                    sc[:],
                                    kT[2 * kv + c][:, ts(t, 128)],
                                    qT[2 * h + c][:, qi0:qi0 + QW],
                                    start=(c == 0), stop=(c == 1))
                            e = epool.tile([128, QW], b16, tag="e", bufs=12)
                            nc.scalar.activation(e[:], sc[:], Act.Exp,
                                                 scale=SCALE)
                            D = qi0 - t * 128
                            if D in MASK_D:
                                m = MASK_D[D]
                                nc.vector.tensor_tensor(
                                    e[:], e[:], masks_sb[:, ts(m, QW)],
                                    Alu.mult)
                            first, last = (t == t_lo), (t == t_hi)
                            for c in range(2):
                                nc.tensor.matmul(
                                    at[c][:],
                                    v_sb[t][:, ts(2 * kv + c, 128)], e[:],
                                    start=first, stop=last)
                            nc.tensor.matmul(sums[:], ones_col[:], e[:],
                                             start=first, stop=last)
                        # normalize: attnT *= broadcast(1/sums)
                        rc = epool.tile([1, QW], f32, tag="rc", bufs=3)
                        nc.vector.reciprocal(rc[:], sums[:])
                        bc = ps_sc.tile([128, QW], f32, tag="sc")
                        nc.tensor.matmul(bc[:], ones_rowf[:], rc[:],
                                         start=True, stop=True)
                        bcs = epool.tile([128, QW], f32, tag="bcs", bufs=3)
                        nc.scalar.activation(bcs[:], bc[:], Act.Copy)
                        for c in range(2):
                            a = apool.tile([128, QW], b16,
                                           tag=f"at{h}_{qb}_{c}")
                            nc.vector.tensor_tensor(
                                a[:], at[c][:], bcs[:], Alu.mult)
                            attnT[h][qb][c] = a

            # ============ phase 3: output projection ===================
            with tc.tile_pool(name="ysb", bufs=4) as ypool, \
                 tc.tile_pool(name="ps_y", bufs=8, space="PSUM") as ps_y:
                for tb16 in range(S // 128):
                    qb, half = tb16 // 2, tb16 % 2
                    yps = [ps_y.tile([128, 512], f32, tag="y", name=f"y{tb16}_{hb}")
                           for hb in range(H // 512)]
                    n_acc = NHC * 2
                    i = 0
                    for h in range(NHC):
                        for c in range(2):
                            lhsT = attnT[h][qb][c][:, ts(half, 128)]
                            for hb in range(H // 512):
                                nc.tensor.matmul(
                                    yps[hb][:], lhsT,
                                    wo_sb[2 * h + c][:, ts(hb, 512)],
                                    start=(i == 0), stop=(i == n_acc - 1))
                            i += 1
                    ysb = ypool.tile([128, H], b16, tag="ysb")
                    for hb in range(H // 512):
                        nc.scalar.activation(
                            ysb[:, ts(hb, 512)], yps[hb][:], Act.Copy)
                    nc.sync.dma_start(y_d[ts(tb16, 128), :], ysb[:])

    nc.compile()
    return nc


def _get_program():
    if "nc" not in _PROGRAM_CACHE:
        _PROGRAM_CACHE["nc"] = _build_program()
    return _PROGRAM_CACHE["nc"]


def _host_masks():
    kjl = np.arange(128)[:, None]
    qil = np.arange(QW)[None, :]
    out = np.zeros((128, 4 * QW), dtype=bf16)
    for D, i in MASK_D.items():
        rel = D + qil - kjl
        out[:, i * QW:(i + 1) * QW] = ((rel >= 0) & (rel < WINDOW)).astype(bf16)
    return out


def make_in_maps(hidden_states, positions, Wq, bq, Wk, bk, Wv, bv, Wo):
    hs = np.asarray(hidden_states, np.float32)
    pos = np.asarray(positions)
    Wq = np.asarray(Wq, np.float32)
    Wk = np.asarray(Wk, np.float32)
    Wv = np.asarray(Wv, np.float32)
    Wo = np.asarray(Wo, np.float32)
    bq = np.asarray(bq, np.float32)
    bk = np.asarray(bk, np.float32)
    bv = np.asarray(bv, np.float32)

    hsT = np.ascontiguousarray(hs.transpose(0, 2, 1)).astype(bf16)  # [B,H,S]
    inv_freq = (1.0 / (ROPE_BASE ** (np.arange(HALF, dtype=np.float32) / HALF)))
    masks = _host_masks()

    in_maps = []
    for c in range(NCORES):
        b, hg = c // 2, c % 2
        freqs = inv_freq[:, None] * pos[b][None, :].astype(np.float32)
        in_maps.append({
            "hsT": hsT[b],
            "wq": np.ascontiguousarray(Wq[:, hg * DQ:(hg + 1) * DQ]).astype(bf16),
            "wk": np.ascontiguousarray(Wk[:, hg * DK:(hg + 1) * DK]).astype(bf16),
            "wv": np.ascontiguousarray(Wv[:, hg * DK:(hg + 1) * DK]).astype(bf16),
            "wo": np.ascontiguousarray(Wo[hg * DQ:(hg + 1) * DQ, :]).astype(bf16),
            "cosT": np.cos(freqs).astype(np.float32),
            "sinT": np.sin(freqs).astype(np.float32),
            "masks": masks,
            "bq": np.ascontiguousarray(
                bq[hg * DQ:(hg + 1) * DQ].reshape(NQD, 128).T).astype(np.float32),
            "bk": np.ascontiguousarray(
                bk[hg * DK:(hg + 1) * DK].reshape(NKD, 128).T).astype(np.float32),
            "bv": bv[hg * DK:(hg + 1) * DK].reshape(1, DK).astype(bf16),
        })
    return in_maps


def combine_outputs(results):
    out = np.empty((B, S, H), np.float32)
    for b in range(B):
        out[b] = (results[2 * b]["y"].astype(np.float32)
                  + results[2 * b + 1]["y"].astype(np.float32))
    return out


def bench(inputs, iters=20, warmup=2):
    """Measure per-execution HW time (ns) by pipelining async dispatches
    against device-resident inputs.  Not used by the grading harness."""
    import time
    _ensure_paths()
    import jax
    import jax.numpy as jnp  # noqa: F401
    from jax.sharding import Mesh, PartitionSpec, NamedSharding
    from jax.experimental.shard_map import shard_map
    import concourse.mybir as mybir
    from concourse import bass2jax

    bass2jax.install_neuronx_cc_hook()
    nc = _get_program()
    in_maps = make_in_maps(**inputs)

    partition_name = (nc.partition_id_tensor.name
                      if nc.partition_id_tensor else None)
    in_names, out_names, out_avals, zero_outs = [], [], [], []
    for alloc in nc.m.functions[0].allocations:
        if not isinstance(alloc, mybir.MemoryLocationSet):
            continue
        name = alloc.memorylocations[0].name
        if alloc.kind == "ExternalInput":
            if name != partition_name:
                in_names.append(name)
        elif alloc.kind == "ExternalOutput":
            out_names.append(name)
            shape = tuple(alloc.tensor_shape)
            dtype = mybir.dt.np(alloc.dtype)
            out_avals.append(jax.core.ShapedArray(shape, dtype))
            zero_outs.append(np.zeros((NCORES * shape[0], *shape[1:]), dtype))
    n_params = len(in_names)

    body_in_names = in_names + out_names
    if partition_name is not None:
        body_in_names = body_in_names + [partition_name]

    def _body(*args):
        operands = list(args)
        if partition_name is not None:
            operands.append(bass2jax.partition_id_tensor())
        outs = bass2jax._bass_exec_p.bind(
            *operands,
            out_avals=tuple(out_avals),
            in_names=tuple(body_in_names),
            out_names=tuple(out_names),
            lowering_input_output_aliases=(),
            sim_require_finite=True,
            sim_require_nnan=True,
            nc=nc,
        )
        return tuple(outs)

    devices = jax.devices()[:NCORES]
    mesh = Mesh(np.asarray(devices), ("core",))
    nshard = NamedSharding(mesh, PartitionSpec("core"))
    n_outs = len(out_names)
    sharded = jax.jit(
        shard_map(_body, mesh=mesh,
                  in_specs=(PartitionSpec("core"),) * (n_params + n_outs),
                  out_specs=(PartitionSpec("core"),) * n_outs,
                  check_rep=False),
        keep_unused=True,
    )
    concat_in = [
        np.concatenate([np.asarray(in_maps[c][name]) for c in range(NCORES)],
                       axis=0)
        for name in in_names
    ]
    dev_args = [jax.device_put(a, nshard) for a in concat_in + zero_outs]

    for _ in range(warmup):
        out = sharded(*dev_args)
    jax.block_until_ready(out)
    t0 = time.perf_counter()
    outs = [sharded(*dev_args) for _ in range(iters)]
    jax.block_until_ready(outs)
    t1 = time.perf_counter()
    return (t1 - t0) / iters * 1e9


def kernel(hidden_states, positions, Wq, bq, Wk, bk, Wv, bv, Wo):
    _ensure_paths()
    from concourse import bass_utils

    nc = _get_program()
    in_maps = make_in_maps(hidden_states, positions, Wq, bq, Wk, bk, Wv, bv, Wo)
    try:
        res = bass_utils.run_bass_kernel_spmd(
            nc, in_maps, core_ids=list(range(NCORES)), trace=False)
    except ModuleNotFoundError:
        os.environ["BASS_NEVER_TRACE"] = "1"
        res = bass_utils.run_bass_kernel_spmd(
            nc, in_maps, core_ids=list(range(NCORES)), trace=False)
    return combine_outputs(res.results)


# revision 9
# speedup vs baseline: 6.8857x; 6.8857x over previous
# Gemma2 sliding-window attention on 8 Trainium2 NeuronCores (Bass/Tile).
#
# Sharding: core c -> (batch = c//2, head-group = c%2).  Each core computes
# 4 q-heads / 2 kv-heads of one batch: QKV projections, RoPE, windowed
# causal attention, and a partial output projection y_part = attn @ Wo[rows
# of its head-group].  The host sums the two partials per batch (Wo is
# split on its input dim) — that is the only cross-core reduction.
#
# On-device layout strategy (all matmuls bf16, accumulation fp32 in PSUM):
#   hsT  [H, S]    host-pre-transposed hidden states (bf16)
#   qT   [dq, S]   = Wq_chunk.T @ hsT   (RoPE fused into PSUM evacuation)
#   kT   [dk, S]   likewise
#   v    [S, dk]   = hsT_chunk.T @ Wv
#   scoresT [kj, qi] = kT_chunk.T @ qT  (transposed scores: feeds PV directly)
#   expT = exp(scoresT * SCALE)         (no running max: |scores*SCALE| < ~7)
#   attnT [d, qi] = v_chunk.T @ expT ; sums[1, qi] = ones.T @ expT
#   attnT normalized by broadcast(1/sums) at PSUM evacuation
#   y [S, H] = attnT_chunk.T @ Wo_chunk
import os
import sys

import numpy as np
import ml_dtypes

bf16 = ml_dtypes.bfloat16

B, S, H = 4, 2048, 2048
NH, NKV, HD = 8, 4, 256
WINDOW = 1024
ROPE_BASE = 10000.0
SCALE = 256.0 ** -0.5
NCORES = 8

NHC, NKVC = NH // 2, NKV // 2      # per-core q heads / kv heads
DQ, DK = NHC * HD, NKVC * HD       # 1024, 512
HALF = HD // 2                     # 128 (rope half-dim == d-chunk)
NQD, NKD = DQ // 128, DK // 128    # q / k d-blocks (8 / 4)
NHCH = H // 128                    # 16 hidden chunks
TB = 512                           # token block for projections
NTB = S // TB                      # 4
QW = 256                           # attention query-block width
NQB = S // QW                      # 8
NKB = S // 128                     # 16 key blocks
# boundary-mask IDs by D = qi0 - kj0
MASK_D = {0: 0, -128: 1, 896: 2, 1024: 3}

_PROGRAM_CACHE = {}


def _ensure_paths():
    try:
        import concourse.bass  # noqa: F401
    except ImportError:
        for p in ("/root/.axon_site/_ro/trn_rl_repo", "/opt/trn_rl_repo"):
            if os.path.isdir(p) and p not in sys.path:
                sys.path.insert(0, p)


def _build_program(reps=1):
    _ensure_paths()
    import concourse.bacc as bacc
    import concourse.bass as bass
    import concourse.mybir as mybir
    import concourse.tile as tile
    from contextlib import ExitStack

    f32 = mybir.dt.float32
    b16 = mybir.dt.bfloat16
    Alu = mybir.AluOpType
    Act = mybir.ActivationFunctionType
    ts = bass.ts

    nc = bacc.Bacc("TRN2", target_bir_lowering=False, debug=False,
                   num_devices=NCORES)

    hsT_d = nc.dram_tensor("hsT", [H, S], b16, kind="ExternalInput").ap()
    wq_d = nc.dram_tensor("wq", [H, DQ], b16, kind="ExternalInput").ap()
    wk_d = nc.dram_tensor("wk", [H, DK], b16, kind="ExternalInput").ap()
    wv_d = nc.dram_tensor("wv", [H, DK], b16, kind="ExternalInput").ap()
    wo_d = nc.dram_tensor("wo", [DQ, H], b16, kind="ExternalInput").ap()
    cos_d = nc.dram_tensor("cosT", [HALF, S], f32, kind="ExternalInput").ap()
    sin_d = nc.dram_tensor("sinT", [HALF, S], f32, kind="ExternalInput").ap()
    masks_d = nc.dram_tensor("masks", [128, 4 * QW], b16, kind="ExternalInput").ap()
    bq_d = nc.dram_tensor("bq", [128, NQD], f32, kind="ExternalInput").ap()
    bk_d = nc.dram_tensor("bk", [128, NKD], f32, kind="ExternalInput").ap()
    bv_d = nc.dram_tensor("bv", [1, DK], b16, kind="ExternalInput").ap()
    y_d = nc.dram_tensor("y", [S, H], b16, kind="ExternalOutput").ap()

    with tile.TileContext(nc) as tc:
      for rep in range(reps):
       with ExitStack() as ctx:
        persist = ctx.enter_context(tc.tile_pool(name="persist", bufs=1))

        # ---- persistent small constants -------------------------------
        masks_sb = persist.tile([128, 4 * QW], b16, tag="masks")
        nc.sync.dma_start(masks_sb[:], masks_d[:])
        bq_sb = persist.tile([128, NQD], f32, tag="bq")
        nc.sync.dma_start(bq_sb[:], bq_d[:])
        bk_sb = persist.tile([128, NKD], f32, tag="bk")
        nc.sync.dma_start(bk_sb[:], bk_d[:])
        bv_sb = persist.tile([1, DK], b16, tag="bv")
        nc.sync.dma_start(bv_sb[:], bv_d[:])
        ones_col = persist.tile([128, 1], b16, tag="ones_col")
        nc.gpsimd.memset(ones_col[:], 1.0)
        ones_rowb = persist.tile([1, 128], b16, tag="ones_rowb")
        nc.gpsimd.memset(ones_rowb[:], 1.0)
        ones_rowf = persist.tile([1, 128], f32, tag="ones_rowf")
        nc.gpsimd.memset(ones_rowf[:], 1.0)

        # ---- persistent activation arrays -----------------------------
        qT = [persist.tile([128, S], b16, tag=f"qT{i}", name=f"qT{i}")
              for i in range(NQD)]
        kT = [persist.tile([128, S], b16, tag=f"kT{i}", name=f"kT{i}")
              for i in range(NKD)]
        v_sb = [persist.tile([128, DK], b16, tag=f"v{i}", name=f"v{i}")
                for i in range(NKB)]

        # ================= phase 1: QKV projections ====================
        with tc.tile_pool(name="qkvw", bufs=1) as wpool, \
             tc.tile_pool(name="hst", bufs=24) as hpool, \
             tc.tile_pool(name="ropetmp", bufs=6) as tpool, \
             tc.tile_pool(name="ps_qkv", bufs=4, space="PSUM") as ps_qkv:

            cos_sb = wpool.tile([HALF, S], f32, tag="cos")
            nc.sync.dma_start(cos_sb[:], cos_d[:])
            sin_sb = wpool.tile([HALF, S], f32, tag="sin")
            nc.sync.dma_start(sin_sb[:], sin_d[:])

            wq_sb, wk_sb, wv_sb = [], [], []
            for hc in range(NHCH):
                t = wpool.tile([128, DQ], b16, tag=f"wq{hc}")
                nc.sync.dma_start(t[:], wq_d[ts(hc, 128), :])
                wq_sb.append(t)
                t = wpool.tile([128, DK], b16, tag=f"wk{hc}")
                nc.sync.dma_start(t[:], wk_d[ts(hc, 128), :])
                wk_sb.append(t)
                t = wpool.tile([128, DK], b16, tag=f"wv{hc}")
                nc.sync.dma_start(t[:], wv_d[ts(hc, 128), :])
                wv_sb.append(t)

            for tb in range(NTB):
                hsT_sb = []
                for hc in range(NHCH):
                    t = hpool.tile([128, TB], b16, tag="hst")
                    nc.sync.dma_start(t[:], hsT_d[ts(hc, 128), ts(tb, TB)])
                    hsT_sb.append(t)

                # q / k projections, d-block pairs, rope on evacuation
                for kind, nd, w_sb, out_sb, b_sb in (
                        ("q", NQD, wq_sb, qT, bq_sb),
                        ("k", NKD, wk_sb, kT, bk_sb)):
                    for dp in range(nd // 2):
                        ps = []
                        for c in range(2):
                            db = 2 * dp + c
                            p = ps_qkv.tile([128, TB], f32, tag="psqkv")
                            for hc in range(NHCH):
                                nc.tensor.matmul(
                                    p[:], w_sb[hc][:, ts(db, 128)],
                                    hsT_sb[hc][:],
                                    start=(hc == 0), stop=(hc == NHCH - 1))
                            ps.append(p)
                        # rope: out1 = (p0+b0)c - (p1+b1)s ; out2 = (p1+b1)c + (p0+b0)s
                        b0 = b_sb[:, 2 * dp:2 * dp + 1]
                        b1 = b_sb[:, 2 * dp + 1:2 * dp + 2]
                        cs = cos_sb[:, ts(tb, TB)]
                        sn = sin_sb[:, ts(tb, TB)]
                        t1 = tpool.tile([128, TB], f32, tag="rt")
                        nc.vector.scalar_tensor_tensor(
                            t1[:], ps[0][:], b0, cs, Alu.add, Alu.mult)
                        t2 = tpool.tile([128, TB], f32, tag="rt")
                        nc.vector.scalar_tensor_tensor(
                            t2[:], ps[1][:], b1, sn, Alu.add, Alu.mult)
                        nc.vector.tensor_sub(
                            out_sb[2 * dp][:, ts(tb, TB)], t1[:], t2[:])
                        t3 = tpool.tile([128, TB], f32, tag="rt")
                        nc.vector.scalar_tensor_tensor(
                            t3[:], ps[1][:], b1, cs, Alu.add, Alu.mult)
                        t4 = tpool.tile([128, TB], f32, tag="rt")
                        nc.vector.scalar_tensor_tensor(
                            t4[:], ps[0][:], b0, sn, Alu.add, Alu.mult)
                        nc.vector.tensor_add(
                            out_sb[2 * dp + 1][:, ts(tb, TB)], t3[:], t4[:])

                # v projection (token-major) + bias row
                for st in range(TB // 128):
                    kb = tb * (TB // 128) + st
                    p = ps_qkv.tile([128, DK], f32, tag="psqkv")
                    for hc in range(NHCH):
                        nc.tensor.matmul(
                            p[:], hsT_sb[hc][:, ts(st, 128)], wv_sb[hc][:],
                            start=(hc == 0), stop=False)
                    nc.tensor.matmul(p[:], ones_rowb[:], bv_sb[:],
                                     start=False, stop=True)
                    nc.scalar.activation(v_sb[kb][:], p[:], Act.Copy)

        # ============ phase 2: windowed attention per head =============
        attnT = [[[None, None] for _ in range(NQB)] for _ in range(NHC)]
        with tc.tile_pool(name="attn", bufs=1) as apool:
            wo_sb = []
            for dc in range(NQD):
                t = apool.tile([128, H], b16, tag=f"wo{dc}")
                nc.sync.dma_start(t[:], wo_d[ts(dc, 128), :])
                wo_sb.append(t)

            with tc.tile_pool(name="expp", bufs=12) as epool, \
                 tc.tile_pool(name="ps_sc", bufs=3, space="PSUM") as ps_sc, \
                 tc.tile_pool(name="ps_at", bufs=3, space="PSUM") as ps_at, \
                 tc.tile_pool(name="ps_sum", bufs=2, space="PSUM") as ps_sum:
                for h in range(NHC):
                    kv = h // 2
                    for qb in range(NQB):
                        qi0 = qb * QW
                        t_lo = max(0, 2 * qb - 8)
                        t_hi = 2 * qb + 1
                        sums = ps_sum.tile([1, QW], f32, tag="sums")
                        at = [ps_at.tile([128, QW], f32, tag="at", name=f"at{h}_{qb}_{c}")
                              for c in range(2)]
                        for t in range(t_lo, t_hi + 1):
                            sc = ps_sc.tile([128, QW], f32, tag="sc")
                            for c in range(2):
                                nc.tensor.matmul(
                                    sc[:],
                                    kT[2 * kv + c][:, ts(t, 128)],
                                    qT[2 * h + c][:, qi0:qi0 + QW],
                                    start=(c == 0), stop=(c == 1))
                            e = epool.tile([128, QW], b16, tag="e", bufs=12)
                            nc.scalar.activation(e[:], sc[:], Act.Exp,
                                                 scale=SCALE)
                            D = qi0 - t * 128
                            if D in MASK_D:
                                m = MASK_D[D]
                                nc.vector.tensor_tensor(
                                    e[:], e[:], masks_sb[:, ts(m, QW)],
                                    Alu.mult)
                            first, last = (t == t_lo), (t == t_hi)
                            for c in range(2):
                                nc.tensor.matmul(
                                    at[c][:],
                                    v_sb[t][:, ts(2 * kv + c, 128)], e[:],
                                    start=first, stop=last)
                            nc.tensor.matmul(sums[:], ones_col[:], e[:],
                                             start=first, stop=last)
                        # normalize: attnT *= broadcast(1/sums)
                        rc = epool.tile([1, QW], f32, tag="rc", bufs=3)
                        nc.vector.reciprocal(rc[:], sums[:])
                        bc = ps_sc.tile([128, QW], f32, tag="sc")
                        nc.tensor.matmul(bc[:], ones_rowf[:], rc[:],
                                         start=True, stop=True)
                        bcs = epool.tile([128, QW], f32, tag="bcs", bufs=3)
                        nc.scalar.activation(bcs[:], bc[:], Act.Copy)
                        for c in range(2):
                            a = apool.tile([128, QW], b16,
                                           tag=f"at{h}_{qb}_{c}")
                            nc.vector.tensor_tensor(
                                a[:], at[c][:], bcs[:], Alu.mult)
                            attnT[h][qb][c] = a

            # ============ phase 3: output projection ===================
            with tc.tile_pool(name="ysb", bufs=4) as ypool, \
                 tc.tile_pool(name="ps_y", bufs=8, space="PSUM") as ps_y:
                for tb16 in range(S // 128):
                    qb, half = tb16 // 2, tb16 % 2
                    yps = [ps_y.tile([128, 512], f32, tag="y", name=f"y{tb16}_{hb}")
                           for hb in range(H // 512)]
                    n_acc = NHC * 2
                    i = 0
                    for h in range(NHC):
                        for c in range(2):
                            lhsT = attnT[h][qb][c][:, ts(half, 128)]
                            for hb in range(H // 512):
                                nc.tensor.matmul(
                                    yps[hb][:], lhsT,
                                    wo_sb[2 * h + c][:, ts(hb, 512)],
                                    start=(i == 0), stop=(i == n_acc - 1))
                            i += 1
                    ysb = ypool.tile([128, H], b16, tag="ysb")
                    for hb in range(H // 512):
                        nc.scalar.activation(
                            ysb[:, ts(hb, 512)], yps[hb][:], Act.Copy)
                    nc.sync.dma_start(y_d[ts(tb16, 128), :], ysb[:])

    nc.compile()
    return nc


def _get_program(reps=1):
    key = ("nc", reps)
    if key not in _PROGRAM_CACHE:
        _PROGRAM_CACHE[key] = _build_program(reps)
    return _PROGRAM_CACHE[key]


def _host_masks():
    kjl = np.arange(128)[:, None]
    qil = np.arange(QW)[None, :]
    out = np.zeros((128, 4 * QW), dtype=bf16)
    for D, i in MASK_D.items():
        rel = D + qil - kjl
        out[:, i * QW:(i + 1) * QW] = ((rel >= 0) & (rel < WINDOW)).astype(bf16)
    return out


def make_in_maps(hidden_states, positions, Wq, bq, Wk, bk, Wv, bv, Wo):
    hs = np.asarray(hidden_states, np.float32)
    pos = np.asarray(positions)
    Wq = np.asarray(Wq, np.float32)
    Wk = np.asarray(Wk, np.float32)
    Wv = np.asarray(Wv, np.float32)
    Wo = np.asarray(Wo, np.float32)
    bq = np.asarray(bq, np.float32)
    bk = np.asarray(bk, np.float32)
    bv = np.asarray(bv, np.float32)

    hsT = np.ascontiguousarray(hs.transpose(0, 2, 1)).astype(bf16)  # [B,H,S]
    inv_freq = (1.0 / (ROPE_BASE ** (np.arange(HALF, dtype=np.float32) / HALF)))
    masks = _host_masks()

    in_maps = []
    for c in range(NCORES):
        b, hg = c // 2, c % 2
        freqs = inv_freq[:, None] * pos[b][None, :].astype(np.float32)
        in_maps.append({
            "hsT": hsT[b],
            "wq": np.ascontiguousarray(Wq[:, hg * DQ:(hg + 1) * DQ]).astype(bf16),
            "wk": np.ascontiguousarray(Wk[:, hg * DK:(hg + 1) * DK]).astype(bf16),
            "wv": np.ascontiguousarray(Wv[:, hg * DK:(hg + 1) * DK]).astype(bf16),
            "wo": np.ascontiguousarray(Wo[hg * DQ:(hg + 1) * DQ, :]).astype(bf16),
            "cosT": np.cos(freqs).astype(np.float32),
            "sinT": np.sin(freqs).astype(np.float32),
            "masks": masks,
            "bq": np.ascontiguousarray(
                bq[hg * DQ:(hg + 1) * DQ].reshape(NQD, 128).T).astype(np.float32),
            "bk": np.ascontiguousarray(
                bk[hg * DK:(hg + 1) * DK].reshape(NKD, 128).T).astype(np.float32),
            "bv": bv[hg * DK:(hg + 1) * DK].reshape(1, DK).astype(bf16),
        })
    return in_maps


def combine_outputs(results):
    out = np.empty((B, S, H), np.float32)
    for b in range(B):
        out[b] = (results[2 * b]["y"].astype(np.float32)
                  + results[2 * b + 1]["y"].astype(np.float32))
    return out


def bench(inputs, iters=20, warmup=2, reps=1):
    """Measure per-execution HW time (ns) by pipelining async dispatches
    against device-resident inputs.  Not used by the grading harness."""
    import time
    _ensure_paths()
    import jax
    import jax.numpy as jnp  # noqa: F401
    from jax.sharding import Mesh, PartitionSpec, NamedSharding
    from jax.experimental.shard_map import shard_map
    import concourse.mybir as mybir
    from concourse import bass2jax

    bass2jax.install_neuronx_cc_hook()
    nc = _get_program(reps)
    in_maps = make_in_maps(**inputs)

    partition_name = (nc.partition_id_tensor.name
                      if nc.partition_id_tensor else None)
    in_names, out_names, out_avals, zero_outs = [], [], [], []
    for alloc in nc.m.functions[0].allocations:
        if not isinstance(alloc, mybir.MemoryLocationSet):
            continue
        name = alloc.memorylocations[0].name
        if alloc.kind == "ExternalInput":
            if name != partition_name:
                in_names.append(name)
        elif alloc.kind == "ExternalOutput":
            out_names.append(name)
            shape = tuple(alloc.tensor_shape)
            dtype = mybir.dt.np(alloc.dtype)
            out_avals.append(jax.core.ShapedArray(shape, dtype))
            zero_outs.append(np.zeros((NCORES * shape[0], *shape[1:]), dtype))
    n_params = len(in_names)

    body_in_names = in_names + out_names
    if partition_name is not None:
        body_in_names = body_in_names + [partition_name]

    def _body(*args):
        operands = list(args)
        if partition_name is not None:
            operands.append(bass2jax.partition_id_tensor())
        outs = bass2jax._bass_exec_p.bind(
            *operands,
            out_avals=tuple(out_avals),
            in_names=tuple(body_in_names),
            out_names=tuple(out_names),
            lowering_input_output_aliases=(),
            sim_require_finite=True,
            sim_require_nnan=True,
            nc=nc,
        )
        return tuple(outs)

    devices = jax.devices()[:NCORES]
    mesh = Mesh(np.asarray(devices), ("core",))
    nshard = NamedSharding(mesh, PartitionSpec("core"))
    n_outs = len(out_names)
    sharded = jax.jit(
        shard_map(_body, mesh=mesh,
                  in_specs=(PartitionSpec("core"),) * (n_params + n_outs),
                  out_specs=(PartitionSpec("core"),) * n_outs,
                  check_rep=False),
        keep_unused=True,
    )
    concat_in = [
        np.concatenate([np.asarray(in_maps[c][name]) for c in range(NCORES)],
                       axis=0)
        for name in in_names
    ]
    dev_args = [jax.device_put(a, nshard) for a in concat_in + zero_outs]

    for _ in range(warmup):
        out = sharded(*dev_args)
    jax.block_until_ready(out)
    t0 = time.perf_counter()
    outs = [sharded(*dev_args) for _ in range(iters)]
    jax.block_until_ready(outs)
    t1 = time.perf_counter()
    return (t1 - t0) / iters * 1e9


def kernel(hidden_states, positions, Wq, bq, Wk, bk, Wv, bv, Wo):
    _ensure_paths()
    from concourse import bass_utils

    nc = _get_program()
    in_maps = make_in_maps(hidden_states, positions, Wq, bq, Wk, bk, Wv, bv, Wo)
    try:
        res = bass_utils.run_bass_kernel_spmd(
            nc, in_maps, core_ids=list(range(NCORES)), trace=False)
    except ModuleNotFoundError:
        os.environ["BASS_NEVER_TRACE"] = "1"
        res = bass_utils.run_bass_kernel_spmd(
            nc, in_maps, core_ids=list(range(NCORES)), trace=False)
    return combine_outputs(res.results)
